# revision 33
# baseline (speedup 1.0000x reference)
"""
Trainium2 Bass kernel for nn_CrossAttention_62027917689453 — fp8 DoubleRow.

Math (per batch b):
    q = rgb @ Wq                       (N, E)
    k = freq @ Wk                      (N, E)
    scores = q @ k.T / sqrt(E)         (N, N)
    attn = softmax(scores, axis=-1)
    attn_out = attn @ freq             (N, D)
    out = concat([rgb, 0.5 * attn_out], axis=-1)   (N, 2D)

(ifreq / Wv are dead inputs in the reference and are ignored.)

Sharding: data-parallel over batch — 8 batches onto 8 NeuronCores.

fp8 scheme (all matmuls float8e4 + perf_mode=DoubleRow, 2 fp8/PE cell,
contraction 256/instruction):
  - Wq/Wk are scaled by 32 on load so their entries are ~N(0,1) (raw
    entries ~N(0, 1/1024) would be subnormal in e4m3).  q', k' then have
    sigma=32 (max ~185 < 240 = TRN e4m3 max).  scores' = 1024 * raw.
  - exp uses scale=1/32768 and bias=-3: P = exp(scores/32 - 3).  The
    constant bias cancels in the softmax normalization and keeps
    max(P) ~ 31 < 240 so the fp8 store of P cannot overflow to Inf.
  - P is stored fp8; the softmax denominator is computed FROM THE SAME
    fp8 P values (ones-stationary DoubleRow matmuls accumulating a
    [1, 512] PSUM row), so numerator/denominator stay consistent.
  - The [1, nblk] column-sum row is moved into [n-partition, 1] layout
    with 4 tiny K=1 matmuls (vector outer product with scalar 1), after
    a reciprocal on DVE.  Normalization multiplies U by rc * 0.5.
  - Scores are computed TRANSPOSED (sT[m, n]) so exp(sT) is directly the
    stationary operand of the attention-output matmul, as in the bf16
    version.  All PE transposes (freqT / rgbT) run on fp8 data.
"""

import numpy as np

import concourse.bass as bass
import concourse.mybir as mybir
import concourse.tile as tile
from concourse.tile import TileContext

from concourse.masks import make_identity

F32 = mybir.dt.float32
F16 = mybir.dt.float16
BF16 = mybir.dt.bfloat16
F8 = mybir.dt.float8e4
DR = mybir.MatmulPerfMode.DoubleRow
EXP = mybir.ActivationFunctionType.Exp

B = 8          # batches == cores
N = 2048       # sequence length (n and m)
D = 1024       # feature dim (d and e)
P = 128        # partitions
NT = N // P    # 16  row chunks
DC = D // P    # 8   feature chunks
PAIR = DC // 2   # 4 DoubleRow contraction steps over d/e
MPAIR = NT // 2  # 8 DoubleRow contraction steps over m
NBLK = 512     # n-block width for the q/scores pipeline
NG = N // NBLK # 4   n-blocks
SUB = NBLK // P  # 4 row-chunks per n-block

WSCALE = 32.0              # Wq/Wk prescale (fp8 dynamic range)
EXP_SCALE = 1.0 / (WSCALE * WSCALE * 32.0)   # recovers scores/sqrt(E)
EXP_BIAS = -3.0            # constant shift, cancels in normalization


def _split_multi_waits(nc: bass.Bass) -> int:
    """The walrus build in this container cannot encode multi-semaphore waits
    on several instruction structs (CTRL Drain, PSEUDO_DMA_DIRECT2D, ...):
    setupSyncWait throws an internal error.  Rewrite every instruction that
    carries more than one wait so the extra waits sit on standalone
    single-wait EventSemaphore instructions immediately before it."""
    n_split = 0
    for f in nc.m.functions:
        for blk in f.blocks:
            insts = blk.instructions
            new: list = []
            changed = False
            for inst in insts:
                si = inst.sync_info
                if si is not None and len(si.on_wait) > 1:
                    waits = list(si.on_wait)
                    for w in waits[:-1]:
                        n_split += 1
                        ev = mybir.InstEventSemaphore(
                            name=f"I-msw-{n_split}",
                            ins=[],
                            outs=[],
                            sync_info=mybir.SyncInfo(on_wait=[w], on_update=[]),
                        )
                        ev.engine = inst.engine
                        new.append(ev)
                    si.on_wait.clear()
                    si.on_wait.append(waits[-1])
                    changed = True
                new.append(inst)
            if changed:
                insts[:] = new
    return n_split


def build_program() -> bass.Bass:
    nc = bass.Bass()
    rgb = nc.declare_dram_parameter("rgb", [N, D], F32, isOutput=False)
    freq = nc.declare_dram_parameter("freq", [N, D], F32, isOutput=False)
    wq = nc.declare_dram_parameter("Wq", [D, D], F32, isOutput=False)
    wk = nc.declare_dram_parameter("Wk", [D, D], F32, isOutput=False)
    out = nc.declare_dram_parameter("out", [N, 2 * D], F32, isOutput=True)

    with TileContext(nc) as tc:
        with (
            tc.tile_pool(name="statics", bufs=1) as statics,
            tc.tile_pool(name="ld", bufs=4) as ldp,
            tc.tile_pool(name="bfp", bufs=2) as bfp,
            tc.tile_pool(name="col", bufs=4) as colp,
            tc.tile_pool(name="qtp", bufs=2) as qtp,
            tc.tile_pool(name="pblk", bufs=2) as pblkp,
            tc.tile_pool(name="outp", bufs=3) as outp,
            tc.tile_pool(name="small", bufs=8) as smallp,
            tc.tile_pool(name="ps", bufs=4, space="PSUM") as psp,
            tc.tile_pool(name="psu", bufs=2, space="PSUM") as psup,
        ):
            ident = statics.tile([P, P], F8, tag="ident")
            make_identity(nc, ident)
            # colsum stationary: ones pair for DoubleRow.  The pair stride
            # (dim1) must be a multiple of 16 bytes, hence the padded shape.
            ones2_t = statics.tile([P, 2, 16], F8, tag="ones2")
            nc.vector.memset(ones2_t, 1.0)
            ones2 = ones2_t[:, :, 0:1]
            # K=1 transpose helper.  Value 2.0: the transposed column sums
            # come out doubled, so their reciprocal is 0.5/colsum — folding
            # the 0.5 fusion weight into the normalization scale for free.
            ones1 = statics.tile([1, 1], F16, tag="ones1")
            nc.vector.memset(ones1, 2.0)
            # per-partition bias column for the exp activation
            ebias = statics.tile([P, 1], F32, tag="ebias")
            nc.vector.memset(ebias, EXP_BIAS)

            wq8 = statics.tile([P, DC, D], F8, tag="wq")
            wk8 = statics.tile([P, DC, D], F8, tag="wk")
            freq8 = statics.tile([P, NT, D], F8, tag="freq8")

            # DMA issue order is the critical-path order: the first PE work
            # (freqT transposes) needs the early freq chunks; kT needs Wk;
            # qT of block 0 needs rgb block 0 + Wq; remaining rgb blocks
            # stream inside the main loop.
            # Input casts are engine-balanced: gpsimd's CAST is ~5x slower
            # than DVE/ScalarE, so it only gets the non-critical prefetched
            # rgb blocks (ng>=1); everything on the prologue critical path
            # alternates between vector and scalar.
            def load_freq(mc):
                t = ldp.tile([P, D], F32, tag="ld")
                nc.sync.dma_start(out=t, in_=freq[mc * P:(mc + 1) * P, :])
                if mc % 2 == 0:
                    nc.vector.tensor_copy(out=freq8[:, mc, :], in_=t)
                else:
                    nc.scalar.copy(out=freq8[:, mc, :], in_=t)

            def load_w(dram, dst, dc):
                t2 = ldp.tile([P, D], F32, tag="ld")
                nc.sync.dma_start(out=t2, in_=dram[dc * P:(dc + 1) * P, :])
                if dc % 2 == 0:
                    nc.vector.tensor_scalar_mul(dst[:, dc, :], t2, WSCALE)
                else:
                    nc.scalar.activation(
                        out=dst[:, dc, :], in_=t2,
                        func=mybir.ActivationFunctionType.Copy, scale=WSCALE,
                    )

            def load_rgb_group(ng, defer_passthrough=False):
                # load rgb chunks; write the rgb passthrough output half.
                # Casts are spread over three engines: a single engine would
                # serialize the group (~3.5us each on gpsimd) and delay the
                # rgbT transposes that gate the next block's scores.
                rgb8 = bfp.tile([P, SUB, D], F8, tag="rgb8",
                                name=f"rgb8_{ng}")
                cast_eng = [nc.vector, nc.scalar, nc.gpsimd, nc.vector]
                fp32_chunks = []
                for s in range(SUB):
                    nchunk = ng * SUB + s
                    t = ldp.tile([P, D], F32, tag="ld")
                    nc.sync.dma_start(
                        out=t, in_=rgb[nchunk * P:(nchunk + 1) * P, :]
                    )
                    eng = nc.vector if ng == 0 else cast_eng[s]
                    if eng is nc.scalar:
                        nc.scalar.copy(out=rgb8[:, s, :], in_=t)
                    else:
                        eng.tensor_copy(out=rgb8[:, s, :], in_=t)
                    if defer_passthrough:
                        fp32_chunks.append(t)
                    else:
                        nc.sync.dma_start(
                            out=out[nchunk * P:(nchunk + 1) * P, 0:D], in_=t
                        )
                return rgb8, fp32_chunks

            # rgb0 interleaved with the first freq group: rcol0's transposes
            # fill the PE hole between ft(0) finishing and Wk arriving
            rgb8_0 = bfp.tile([P, SUB, D], F8, tag="rgb8", name="rgb8_0")
            rgb0_chunks = []
            for mc in range(4):
                load_freq(mc)
                t = ldp.tile([P, D], F32, tag="ld")
                nc.sync.dma_start(out=t, in_=rgb[mc * P:(mc + 1) * P, :])
                nc.vector.tensor_copy(out=rgb8_0[:, mc, :], in_=t)
                rgb0_chunks.append(t)
            for dc in range(DC):
                load_w(wk, wk8, dc)
            for mc in range(4, NT):
                load_freq(mc)
            for dc in range(DC):
                load_w(wq, wq8, dc)

            # ng=0 passthrough writes issue after the critical-path loads
            for s, t in enumerate(rgb0_chunks):
                nc.sync.dma_start(out=out[s * P:(s + 1) * P, 0:D], in_=t)

            # --- kT[e, m] = Wk'[d, e]^T  freqT[d, m]  (all m up front) ---
            kt8 = statics.tile([P, DC, N], F8, tag="kt")
            fcols = [None] * NG

            def emit_ft(mg):
                fcol = colp.tile([P, DC, NBLK], F8, tag="col")
                for dc in range(DC):
                    # fp8 transpose results land in 2-byte cells (walrus:
                    # "FP8 transpose mode must have output element step of
                    # 2"), so the PSUM staging tile is [P, NBLK, 2] and the
                    # drain copy reads the even bytes.
                    ps_t = psp.tile([P, NBLK, 2], F8, tag="ps")
                    for s in range(SUB):
                        mc = mg * SUB + s
                        nc.tensor.transpose(
                            ps_t[:, s * P:(s + 1) * P, 0],
                            freq8[:, mc, dc * P:(dc + 1) * P],
                            ident,
                        )
                    nc.vector.tensor_copy(out=fcol[:, dc, :],
                                          in_=ps_t[:, :, 0])
                fcols[mg] = fcol

            def emit_kt(mg):
                # pair-outer accumulation: all 8 PSUM banks hold one et-tile
                # accumulator each, so kT matmuls start as soon as the first
                # Wk pair is resident.
                fcol = fcols[mg]
                acc_a = psup.tile([P, D], F32, tag="psu")
                acc_b = psup.tile([P, D], F32, tag="psu")
                accs = [
                    acc_a[:, 0:NBLK], acc_a[:, NBLK:D],
                    acc_b[:, 0:NBLK], acc_b[:, NBLK:D],
                ] + [
                    psp.tile([P, NBLK], F32, tag="ps", name=f"kt_acc_{mg}_{j}")
                    for j in range(4)
                ]
                for c in range(PAIR):
                    for et in range(DC):
                        nc.tensor.matmul(
                            accs[et],
                            wk8[:, 2 * c:2 * c + 2, et * P:(et + 1) * P],
                            fcol[:, 2 * c:2 * c + 2, :],
                            start=(c == 0),
                            stop=(c == PAIR - 1),
                            perf_mode=DR,
                        )
                for et in range(DC):
                    dst = kt8[:, et, mg * NBLK:(mg + 1) * NBLK]
                    if et % 2 == 0:
                        nc.scalar.copy(out=dst, in_=accs[et])
                    else:
                        nc.vector.tensor_copy(out=dst, in_=accs[et])

            # --- per-n-block building blocks ---
            def emit_rcol(rgb8, nm):
                # rgbT columns for an n-block (PE transposes)
                rcol = colp.tile([P, DC, NBLK], F8, tag="col",
                                 name=f"rcol_{nm}")
                for dc in range(DC):
                    ps_t = psp.tile([P, NBLK, 2], F8, tag="ps",
                                    name=f"ps_t_{nm}_{dc}")
                    for s in range(SUB):
                        nc.tensor.transpose(
                            ps_t[:, s * P:(s + 1) * P, 0],
                            rgb8[:, s, dc * P:(dc + 1) * P],
                            ident,
                        )
                    nc.vector.tensor_copy(out=rcol[:, dc, :],
                                          in_=ps_t[:, :, 0])
                return rcol

            def emit_qproj(rcol, nm):
                qt = qtp.tile([P, DC, NBLK], F8, tag="qt", name=f"qt_{nm}")
                for et in range(DC):
                    ps_q = psp.tile([P, NBLK], F32, tag="ps",
                                    name=f"ps_q_{nm}_{et}")
                    for c in range(PAIR):
                        nc.tensor.matmul(
                            ps_q,
                            wq8[:, 2 * c:2 * c + 2, et * P:(et + 1) * P],
                            rcol[:, 2 * c:2 * c + 2, :],
                            start=(c == 0),
                            stop=(c == PAIR - 1),
                            perf_mode=DR,
                        )
                    if et % 2 == 0:
                        nc.scalar.copy(out=qt[:, et, :], in_=ps_q)
                    else:
                        nc.vector.tensor_copy(out=qt[:, et, :], in_=ps_q)
                return qt

            def emit_scores(qt, p_blk, aux, nm):
                # scoresT[m, nblk] -> P = exp(scoresT/32768 - 3), fp8.
                # Column sums of P accumulate into aux[0:1, :] via
                # ones-stationary DoubleRow matmuls as pairs complete.
                for mt in range(NT):
                    ps_s = psp.tile([P, NBLK], F32, tag="ps",
                                    name=f"ps_s_{nm}_{mt}")
                    for c in range(PAIR):
                        nc.tensor.matmul(
                            ps_s,
                            kt8[:, 2 * c:2 * c + 2, mt * P:(mt + 1) * P],
                            qt[:, 2 * c:2 * c + 2, :],
                            start=(c == 0),
                            stop=(c == PAIR - 1),
                            perf_mode=DR,
                        )
                    nc.scalar.activation(
                        out=p_blk[:, mt, :],
                        in_=ps_s,
                        func=EXP,
                        scale=EXP_SCALE,
                        bias=ebias[:, 0:1],
                    )
                    if mt % 2 == 1:
                        c = mt // 2
                        nc.tensor.matmul(
                            aux[0:1, 0:NBLK],
                            ones2,
                            p_blk[:, mt - 1:mt + 1, :],
                            start=(c == 0),
                            stop=(c == MPAIR - 1),
                            perf_mode=DR,
                        )

            # --- prologue PE pipeline: transposes of group mg+1 are emitted
            # before the kT matmuls of group mg, so the PE has transpose work
            # while Wk is still loading ---
            emit_ft(0)
            rcol0 = emit_rcol(rgb8_0, 0)
            emit_ft(1)
            emit_kt(0)
            emit_ft(2)
            emit_kt(1)
            emit_ft(3)
            emit_kt(2)
            emit_kt(3)
            qt_cur = emit_qproj(rcol0, 0)

            for ng in range(NG):
                p_blk = pblkp.tile([P, NT, NBLK], F8, tag="pblk",
                                   name=f"pblk_{ng}")
                # aux bank: [0:1, 0:NBLK] holds the colsum row; cols
                # RCOL0..RCOL0+3 hold the transposed reciprocals
                RCOL0 = NBLK - SUB
                aux = psp.tile([P, NBLK], F32, tag="ps", name=f"aux_{ng}")
                emit_scores(qt_cur, p_blk, aux, ng)

                # prefetch + transpose + project the NEXT n-block's q before
                # the long U phase, so the PE never stalls at the boundary
                if ng + 1 < NG:
                    rgb8_next = load_rgb_group(ng + 1)[0]
                    rcol_next = emit_rcol(rgb8_next, ng + 1)
                    qt_cur = emit_qproj(rcol_next, ng + 1)

                # transpose the colsum row into [n-partition, 1] layout via
                # 4 K=1 matmuls (fp16), THEN take the reciprocal in [P, SUB]
                # layout — a [1, 512] reciprocal runs on a single DVE lane
                # (~3.3us); the transposed [128, 4] one is ~100ns.
                cs_row = smallp.tile([1, NBLK], F16, tag="csrow",
                                     name=f"cs_row_{ng}")
                # on DVE: ScalarE's strict 8-deep FIFO is clogged with the
                # next block's exps, which would delay the rc chain and
                # stall the final normalizes
                nc.vector.tensor_copy(out=cs_row, in_=aux[0:1, 0:NBLK])
                for ntl in range(SUB):
                    nc.tensor.matmul(
                        aux[:, RCOL0 + ntl:RCOL0 + ntl + 1],
                        cs_row[0:1, ntl * P:(ntl + 1) * P],
                        ones1,
                        start=True,
                        stop=True,
                    )
                rc_sb = smallp.tile([P, SUB], F32, tag="rcsb",
                                    name=f"rc_sb_{ng}")
                nc.vector.reciprocal(rc_sb, aux[:, RCOL0:RCOL0 + SUB])

                # U[n, d]: DoubleRow accumulation over m pairs
                for ntl in range(SUB):
                    n0 = ntl * P
                    ps_u = psup.tile([P, D], F32, tag="psu")
                    for c in range(MPAIR):
                        lhs = p_blk[:, 2 * c:2 * c + 2, n0:n0 + P]
                        nc.tensor.matmul(
                            ps_u[:, 0:NBLK], lhs,
                            freq8[:, 2 * c:2 * c + 2, 0:NBLK],
                            start=(c == 0), stop=(c == MPAIR - 1),
                            perf_mode=DR,
                        )
                        nc.tensor.matmul(
                            ps_u[:, NBLK:D], lhs,
                            freq8[:, 2 * c:2 * c + 2, NBLK:D],
                            start=(c == 0), stop=(c == MPAIR - 1),
                            perf_mode=DR,
                        )
                    ot = outp.tile([P, D], F32, tag="ot")
                    # out = U * (0.5/colsum); on ScalarE, which is idle
                    # during the U phase (exps are done) — the sooner the
                    # normalize reads ps_u, the sooner the PSUM bank recycles
                    # for the next ntl's matmuls
                    nc.scalar.activation(
                        out=ot, in_=ps_u,
                        func=mybir.ActivationFunctionType.Copy,
                        scale=rc_sb[:, ntl:ntl + 1],
                    )
                    row0 = ng * NBLK + n0
                    nc.sync.dma_start(out=out[row0:row0 + P, D:2 * D], in_=ot)

    _split_multi_waits(nc)
    return nc


_CACHE: dict = {}


def _get_program() -> bass.Bass:
    if "nc" not in _CACHE:
        _CACHE["nc"] = build_program()
    return _CACHE["nc"]


def _run(in_maps, trace=False, **kw):
    from concourse.bass_utils import run_bass_kernel_spmd

    nc = _get_program()
    return run_bass_kernel_spmd(nc, in_maps, list(range(B)), trace=trace, **kw)


def kernel(rgb, freq, ifreq=None, Wq=None, Wk=None, Wv=None, **_unused):
    rgb = np.asarray(rgb, dtype=np.float32)
    freq = np.asarray(freq, dtype=np.float32)
    Wq = np.ascontiguousarray(np.asarray(Wq, dtype=np.float32))
    Wk = np.ascontiguousarray(np.asarray(Wk, dtype=np.float32))
    in_maps = [
        {
            "rgb": np.ascontiguousarray(rgb[c]),
            "freq": np.ascontiguousarray(freq[c]),
            "Wq": Wq,
            "Wk": Wk,
        }
        for c in range(B)
    ]
    res = _run(in_maps, trace=False)
    return np.stack([res.results[c]["out"] for c in range(B)], axis=0)


# revision 36
# speedup vs baseline: 1.0534x; 1.0534x over previous
"""
Trainium2 Bass kernel for nn_CrossAttention_62027917689453 — fp8 DoubleRow.

Math (per batch b):
    q = rgb @ Wq                       (N, E)
    k = freq @ Wk                      (N, E)
    scores = q @ k.T / sqrt(E)         (N, N)
    attn = softmax(scores, axis=-1)
    attn_out = attn @ freq             (N, D)
    out = concat([rgb, 0.5 * attn_out], axis=-1)   (N, 2D)

(ifreq / Wv are dead inputs in the reference and are ignored.)

Sharding: data-parallel over batch — 8 batches onto 8 NeuronCores.

fp8 scheme (all matmuls float8e4 + perf_mode=DoubleRow, 2 fp8/PE cell,
contraction 256/instruction):
  - Wq/Wk are scaled by 32 on load so their entries are ~N(0,1) (raw
    entries ~N(0, 1/1024) would be subnormal in e4m3).  q', k' then have
    sigma=32 (max ~185 < 240 = TRN e4m3 max).  scores' = 1024 * raw.
  - exp uses scale=1/32768 and bias=-3: P = exp(scores/32 - 3).  The
    constant bias cancels in the softmax normalization and keeps
    max(P) ~ 31 < 240 so the fp8 store of P cannot overflow to Inf.
  - P is stored fp8; the softmax denominator is computed FROM THE SAME
    fp8 P values (ones-stationary DoubleRow matmuls accumulating a
    [1, 512] PSUM row), so numerator/denominator stay consistent.
  - The [1, nblk] column-sum row is moved into [n-partition, 1] layout
    with 4 tiny K=1 matmuls (vector outer product with scalar 1), after
    a reciprocal on DVE.  Normalization multiplies U by rc * 0.5.
  - Scores are computed TRANSPOSED (sT[m, n]) so exp(sT) is directly the
    stationary operand of the attention-output matmul, as in the bf16
    version.  All PE transposes (freqT / rgbT) run on fp8 data.
"""

import numpy as np

import concourse.bass as bass
import concourse.mybir as mybir
import concourse.tile as tile
from concourse.tile import TileContext

from concourse.masks import make_identity

F32 = mybir.dt.float32
F16 = mybir.dt.float16
BF16 = mybir.dt.bfloat16
F8 = mybir.dt.float8e4
DR = mybir.MatmulPerfMode.DoubleRow
EXP = mybir.ActivationFunctionType.Exp

B = 8          # batches == cores
N = 2048       # sequence length (n and m)
D = 1024       # feature dim (d and e)
P = 128        # partitions
NT = N // P    # 16  row chunks
DC = D // P    # 8   feature chunks
PAIR = DC // 2   # 4 DoubleRow contraction steps over d/e
MPAIR = NT // 2  # 8 DoubleRow contraction steps over m
NBLK = 512     # n-block width for the q/scores pipeline
NG = N // NBLK # 4   n-blocks
SUB = NBLK // P  # 4 row-chunks per n-block

WSCALE = 32.0              # Wq/Wk prescale (fp8 dynamic range)
EXP_SCALE = 1.0 / (WSCALE * WSCALE * 32.0)   # recovers scores/sqrt(E)
EXP_BIAS = -3.0            # constant shift, cancels in normalization


def _split_multi_waits(nc: bass.Bass) -> int:
    """The walrus build in this container cannot encode multi-semaphore waits
    on several instruction structs (CTRL Drain, PSEUDO_DMA_DIRECT2D, ...):
    setupSyncWait throws an internal error.  Rewrite every instruction that
    carries more than one wait so the extra waits sit on standalone
    single-wait EventSemaphore instructions immediately before it."""
    n_split = 0
    for f in nc.m.functions:
        for blk in f.blocks:
            insts = blk.instructions
            new: list = []
            changed = False
            for inst in insts:
                si = inst.sync_info
                if si is not None and len(si.on_wait) > 1:
                    waits = list(si.on_wait)
                    for w in waits[:-1]:
                        n_split += 1
                        ev = mybir.InstEventSemaphore(
                            name=f"I-msw-{n_split}",
                            ins=[],
                            outs=[],
                            sync_info=mybir.SyncInfo(on_wait=[w], on_update=[]),
                        )
                        ev.engine = inst.engine
                        new.append(ev)
                    si.on_wait.clear()
                    si.on_wait.append(waits[-1])
                    changed = True
                new.append(inst)
            if changed:
                insts[:] = new
    return n_split


def build_program() -> bass.Bass:
    nc = bass.Bass()
    rgb = nc.declare_dram_parameter("rgb", [N, D], F32, isOutput=False)
    freq = nc.declare_dram_parameter("freq", [N, D], F32, isOutput=False)
    wq = nc.declare_dram_parameter("Wq", [D, D], F32, isOutput=False)
    wk = nc.declare_dram_parameter("Wk", [D, D], F32, isOutput=False)
    out = nc.declare_dram_parameter("out", [N, 2 * D], F32, isOutput=True)

    with TileContext(nc) as tc:
        with (
            tc.tile_pool(name="statics", bufs=1) as statics,
            tc.tile_pool(name="ld", bufs=4) as ldp,
            tc.tile_pool(name="bfp", bufs=2) as bfp,
            tc.tile_pool(name="col", bufs=2) as colp,
            tc.tile_pool(name="qtp", bufs=2) as qtp,
            tc.tile_pool(name="pblk", bufs=2) as pblkp,
            tc.tile_pool(name="outp", bufs=3) as outp,
            tc.tile_pool(name="small", bufs=8) as smallp,
            tc.tile_pool(name="ps", bufs=4, space="PSUM") as psp,
            tc.tile_pool(name="psu", bufs=2, space="PSUM") as psup,
        ):
            ident = statics.tile([P, P], F8, tag="ident")
            make_identity(nc, ident)
            # colsum stationary: ones pair for DoubleRow.  The pair stride
            # (dim1) must be a multiple of 16 bytes, hence the padded shape.
            ones2_t = statics.tile([P, 2, 16], F8, tag="ones2")
            nc.vector.memset(ones2_t, 1.0)
            ones2 = ones2_t[:, :, 0:1]
            # K=1 transpose helper.  Value 2.0: the transposed column sums
            # come out doubled, so their reciprocal is 0.5/colsum — folding
            # the 0.5 fusion weight into the normalization scale for free.
            ones1 = statics.tile([1, 1], F16, tag="ones1")
            nc.vector.memset(ones1, 2.0)
            # per-partition bias column for the exp activation
            ebias = statics.tile([P, 1], F32, tag="ebias")
            nc.vector.memset(ebias, EXP_BIAS)

            wq8 = statics.tile([P, DC, D], F8, tag="wq")
            wk8 = statics.tile([P, DC, D], F8, tag="wk")
            freq8 = statics.tile([P, NT, D], F8, tag="freq8")

            # DMA issue order is the critical-path order: the first PE work
            # (freqT transposes) needs the early freq chunks; kT needs Wk;
            # qT of block 0 needs rgb block 0 + Wq; remaining rgb blocks
            # stream inside the main loop.
            # Input casts are engine-balanced: gpsimd's CAST is ~5x slower
            # than DVE/ScalarE, so it only gets the non-critical prefetched
            # rgb blocks (ng>=1); everything on the prologue critical path
            # alternates between vector and scalar.
            def load_freq(mc):
                t = ldp.tile([P, D], F32, tag="ld")
                nc.sync.dma_start(out=t, in_=freq[mc * P:(mc + 1) * P, :])
                if mc % 2 == 0:
                    nc.vector.tensor_copy(out=freq8[:, mc, :], in_=t)
                else:
                    nc.scalar.copy(out=freq8[:, mc, :], in_=t)

            def load_w(dram, dst, dc):
                t2 = ldp.tile([P, D], F32, tag="ld")
                nc.sync.dma_start(out=t2, in_=dram[dc * P:(dc + 1) * P, :])
                if dc % 2 == 0:
                    nc.vector.tensor_scalar_mul(dst[:, dc, :], t2, WSCALE)
                else:
                    nc.scalar.activation(
                        out=dst[:, dc, :], in_=t2,
                        func=mybir.ActivationFunctionType.Copy, scale=WSCALE,
                    )

            def load_rgb_group(ng, defer_passthrough=False):
                # load rgb chunks; write the rgb passthrough output half.
                # Casts are spread over three engines: a single engine would
                # serialize the group (~3.5us each on gpsimd) and delay the
                # rgbT transposes that gate the next block's scores.
                rgb8 = bfp.tile([P, SUB, D], F8, tag="rgb8",
                                name=f"rgb8_{ng}")
                cast_eng = [nc.vector, nc.scalar, nc.gpsimd, nc.vector]
                fp32_chunks = []
                for s in range(SUB):
                    nchunk = ng * SUB + s
                    t = ldp.tile([P, D], F32, tag="ld")
                    nc.sync.dma_start(
                        out=t, in_=rgb[nchunk * P:(nchunk + 1) * P, :]
                    )
                    eng = nc.vector if ng == 0 else cast_eng[s]
                    if eng is nc.scalar:
                        nc.scalar.copy(out=rgb8[:, s, :], in_=t)
                    else:
                        eng.tensor_copy(out=rgb8[:, s, :], in_=t)
                    if defer_passthrough:
                        fp32_chunks.append(t)
                    else:
                        nc.sync.dma_start(
                            out=out[nchunk * P:(nchunk + 1) * P, 0:D], in_=t
                        )
                return rgb8, fp32_chunks

            for mc in range(4):
                load_freq(mc)
            for dc in range(DC):
                load_w(wk, wk8, dc)
            for mc in range(4, NT):
                load_freq(mc)
            rgb8_0, rgb0_chunks = load_rgb_group(0, defer_passthrough=True)
            for dc in range(DC):
                load_w(wq, wq8, dc)

            # ng=0 passthrough writes issue after the critical-path loads
            for s, t in enumerate(rgb0_chunks):
                nc.sync.dma_start(out=out[s * P:(s + 1) * P, 0:D], in_=t)

            # --- kT[e, m] = Wk'[d, e]^T  freqT[d, m]  (all m up front) ---
            kt8 = statics.tile([P, DC, N], F8, tag="kt")
            fcols = [None] * NG

            def emit_ft(mg):
                fcol = colp.tile([P, DC, NBLK], F8, tag="col")
                for dc in range(DC):
                    # fp8 transpose results land in 2-byte cells (walrus:
                    # "FP8 transpose mode must have output element step of
                    # 2"), so the PSUM staging tile is [P, NBLK, 2] and the
                    # drain copy reads the even bytes.
                    ps_t = psp.tile([P, NBLK, 2], F8, tag="ps")
                    for s in range(SUB):
                        mc = mg * SUB + s
                        nc.tensor.transpose(
                            ps_t[:, s * P:(s + 1) * P, 0],
                            freq8[:, mc, dc * P:(dc + 1) * P],
                            ident,
                        )
                    nc.vector.tensor_copy(out=fcol[:, dc, :],
                                          in_=ps_t[:, :, 0])
                fcols[mg] = fcol

            def emit_kt(mg):
                # pair-outer accumulation: all 8 PSUM banks hold one et-tile
                # accumulator each, so kT matmuls start as soon as the first
                # Wk pair is resident.
                fcol = fcols[mg]
                acc_a = psup.tile([P, D], F32, tag="psu")
                acc_b = psup.tile([P, D], F32, tag="psu")
                accs = [
                    acc_a[:, 0:NBLK], acc_a[:, NBLK:D],
                    acc_b[:, 0:NBLK], acc_b[:, NBLK:D],
                ] + [
                    psp.tile([P, NBLK], F32, tag="ps", name=f"kt_acc_{mg}_{j}")
                    for j in range(4)
                ]
                for c in range(PAIR):
                    for et in range(DC):
                        nc.tensor.matmul(
                            accs[et],
                            wk8[:, 2 * c:2 * c + 2, et * P:(et + 1) * P],
                            fcol[:, 2 * c:2 * c + 2, :],
                            start=(c == 0),
                            stop=(c == PAIR - 1),
                            perf_mode=DR,
                        )
                for et in range(DC):
                    dst = kt8[:, et, mg * NBLK:(mg + 1) * NBLK]
                    if et % 2 == 0:
                        nc.scalar.copy(out=dst, in_=accs[et])
                    else:
                        nc.vector.tensor_copy(out=dst, in_=accs[et])

            # --- per-n-block building blocks ---
            def emit_rcol(rgb8, nm):
                # rgbT columns for an n-block (PE transposes)
                rcol = colp.tile([P, DC, NBLK], F8, tag="col",
                                 name=f"rcol_{nm}")
                for dc in range(DC):
                    ps_t = psp.tile([P, NBLK, 2], F8, tag="ps",
                                    name=f"ps_t_{nm}_{dc}")
                    for s in range(SUB):
                        nc.tensor.transpose(
                            ps_t[:, s * P:(s + 1) * P, 0],
                            rgb8[:, s, dc * P:(dc + 1) * P],
                            ident,
                        )
                    nc.vector.tensor_copy(out=rcol[:, dc, :],
                                          in_=ps_t[:, :, 0])
                return rcol

            def emit_qproj(rcol, nm):
                qt = qtp.tile([P, DC, NBLK], F8, tag="qt", name=f"qt_{nm}")
                for et in range(DC):
                    ps_q = psp.tile([P, NBLK], F32, tag="ps",
                                    name=f"ps_q_{nm}_{et}")
                    for c in range(PAIR):
                        nc.tensor.matmul(
                            ps_q,
                            wq8[:, 2 * c:2 * c + 2, et * P:(et + 1) * P],
                            rcol[:, 2 * c:2 * c + 2, :],
                            start=(c == 0),
                            stop=(c == PAIR - 1),
                            perf_mode=DR,
                        )
                    if et % 2 == 0:
                        nc.scalar.copy(out=qt[:, et, :], in_=ps_q)
                    else:
                        nc.vector.tensor_copy(out=qt[:, et, :], in_=ps_q)
                return qt

            def emit_scores(qt, p_blk, aux, nm):
                # scoresT[m, nblk] -> P = exp(scoresT/32768 - 3), fp8.
                # Column sums of P accumulate into aux[0:1, :] via
                # ones-stationary DoubleRow matmuls as pairs complete.
                for mt in range(NT):
                    ps_s = psp.tile([P, NBLK], F32, tag="ps",
                                    name=f"ps_s_{nm}_{mt}")
                    for c in range(PAIR):
                        nc.tensor.matmul(
                            ps_s,
                            kt8[:, 2 * c:2 * c + 2, mt * P:(mt + 1) * P],
                            qt[:, 2 * c:2 * c + 2, :],
                            start=(c == 0),
                            stop=(c == PAIR - 1),
                            perf_mode=DR,
                        )
                    nc.scalar.activation(
                        out=p_blk[:, mt, :],
                        in_=ps_s,
                        func=EXP,
                        scale=EXP_SCALE,
                        bias=ebias[:, 0:1],
                    )
                    if mt % 2 == 1:
                        c = mt // 2
                        nc.tensor.matmul(
                            aux[0:1, 0:NBLK],
                            ones2,
                            p_blk[:, mt - 1:mt + 1, :],
                            start=(c == 0),
                            stop=(c == MPAIR - 1),
                            perf_mode=DR,
                        )

            # --- prologue PE pipeline: transposes of group mg+1 are emitted
            # before the kT matmuls of group mg, so the PE has transpose work
            # while Wk is still loading ---
            emit_ft(0)
            emit_ft(1)
            emit_kt(0)
            emit_ft(2)
            emit_kt(1)
            emit_ft(3)
            emit_kt(2)
            emit_kt(3)
            rcol0 = emit_rcol(rgb8_0, 0)
            qt_cur = emit_qproj(rcol0, 0)

            for ng in range(NG):
                p_blk = pblkp.tile([P, NT, NBLK], F8, tag="pblk",
                                   name=f"pblk_{ng}")
                # aux bank: [0:1, 0:NBLK] holds the colsum row; cols
                # RCOL0..RCOL0+3 hold the transposed reciprocals
                RCOL0 = NBLK - SUB
                aux = psp.tile([P, NBLK], F32, tag="ps", name=f"aux_{ng}")
                emit_scores(qt_cur, p_blk, aux, ng)

                # prefetch + transpose + project the NEXT n-block's q before
                # the long U phase, so the PE never stalls at the boundary
                if ng + 1 < NG:
                    rgb8_next = load_rgb_group(ng + 1)[0]
                    rcol_next = emit_rcol(rgb8_next, ng + 1)
                    qt_cur = emit_qproj(rcol_next, ng + 1)

                # transpose the colsum row into [n-partition, 1] layout via
                # 4 K=1 matmuls (fp16), THEN take the reciprocal in [P, SUB]
                # layout — a [1, 512] reciprocal runs on a single DVE lane
                # (~3.3us); the transposed [128, 4] one is ~100ns.
                cs_row = smallp.tile([1, NBLK], F16, tag="csrow",
                                     name=f"cs_row_{ng}")
                # on DVE: ScalarE's strict 8-deep FIFO is clogged with the
                # next block's exps, which would delay the rc chain and
                # stall the final normalizes
                nc.vector.tensor_copy(out=cs_row, in_=aux[0:1, 0:NBLK])
                for ntl in range(SUB):
                    nc.tensor.matmul(
                        aux[:, RCOL0 + ntl:RCOL0 + ntl + 1],
                        cs_row[0:1, ntl * P:(ntl + 1) * P],
                        ones1,
                        start=True,
                        stop=True,
                    )
                rc_sb = smallp.tile([P, SUB], F32, tag="rcsb",
                                    name=f"rc_sb_{ng}")
                nc.vector.reciprocal(rc_sb, aux[:, RCOL0:RCOL0 + SUB])

                # U[n, d]: DoubleRow accumulation over m pairs
                for ntl in range(SUB):
                    n0 = ntl * P
                    ps_u = psup.tile([P, D], F32, tag="psu")
                    for c in range(MPAIR):
                        lhs = p_blk[:, 2 * c:2 * c + 2, n0:n0 + P]
                        nc.tensor.matmul(
                            ps_u[:, 0:NBLK], lhs,
                            freq8[:, 2 * c:2 * c + 2, 0:NBLK],
                            start=(c == 0), stop=(c == MPAIR - 1),
                            perf_mode=DR,
                        )
                        nc.tensor.matmul(
                            ps_u[:, NBLK:D], lhs,
                            freq8[:, 2 * c:2 * c + 2, NBLK:D],
                            start=(c == 0), stop=(c == MPAIR - 1),
                            perf_mode=DR,
                        )
                    ot = outp.tile([P, D], F32, tag="ot")
                    # out = U * (0.5/colsum); on ScalarE, which is idle
                    # during the U phase (exps are done) — the sooner the
                    # normalize reads ps_u, the sooner the PSUM bank recycles
                    # for the next ntl's matmuls
                    nc.scalar.activation(
                        out=ot, in_=ps_u,
                        func=mybir.ActivationFunctionType.Copy,
                        scale=rc_sb[:, ntl:ntl + 1],
                    )
                    row0 = ng * NBLK + n0
                    nc.sync.dma_start(out=out[row0:row0 + P, D:2 * D], in_=ot)

    _split_multi_waits(nc)
    return nc


_CACHE: dict = {}


def _get_program() -> bass.Bass:
    if "nc" not in _CACHE:
        _CACHE["nc"] = build_program()
    return _CACHE["nc"]


def _run(in_maps, trace=False, **kw):
    from concourse.bass_utils import run_bass_kernel_spmd

    nc = _get_program()
    return run_bass_kernel_spmd(nc, in_maps, list(range(B)), trace=trace, **kw)


def kernel(rgb, freq, ifreq=None, Wq=None, Wk=None, Wv=None, **_unused):
    rgb = np.asarray(rgb, dtype=np.float32)
    freq = np.asarray(freq, dtype=np.float32)
    Wq = np.ascontiguousarray(np.asarray(Wq, dtype=np.float32))
    Wk = np.ascontiguousarray(np.asarray(Wk, dtype=np.float32))
    in_maps = [
        {
            "rgb": np.ascontiguousarray(rgb[c]),
            "freq": np.ascontiguousarray(freq[c]),
            "Wq": Wq,
            "Wk": Wk,
        }
        for c in range(B)
    ]
    res = _run(in_maps, trace=False)
    return np.stack([res.results[c]["out"] for c in range(B)], axis=0)
